# revision 19
# baseline (speedup 1.0000x reference)
"""Trainium2 Bass kernel for AttentionSocialPooling.

Strategy (8 cores, data parallel over batch B=8; core m handles batch b=m):
For each (b,t) the N x N pairwise attention MLP is decomposed as
  hidden[i,j,a] = relu(u[i,a] + v[j,a]),  u = pos@(W1p-W1d)+b1, v = pos@W1d
(channels scaled by |W2[a]|/2 and permuted so positive-W2 channels come
first; column order is channel-blocked: all positive-channel columns
(i-major) then all negative ones, so the two signed reductions read
contiguous memory).  H[j, cols] comes from one PE matmul in fp8e4m3
DoubleRow mode (4-way hi/lo cascade of u and v for ~fp16 accuracy at 2
columns/cycle).  relu on ACT (fp16 out); signed channel reduction = two DVE
tensor_reduces; pre-subtraction and att*mask on GPSIMD; sigmoid on ACT
(scale=2 undoes the 1/2 prescale) batched over 4 timesteps; dist^2 via a
small fp16 matmul; mask = (d^2 < R^2)*offdiag as one DVE op; final row sums
via PE matmuls with w^T / mask^T stationary accumulating into one persistent
PSUM bank, postprocessed once at the end.
"""

import numpy as np
import ml_dtypes

B, T, N, C, A = 8, 64, 128, 2, 16
R2 = 2500.0

bf16 = ml_dtypes.bfloat16
f8 = ml_dtypes.float8_e4m3fn

_CACHE = {}


def _f8_cascade(x, levels=4):
    """Split x into `levels` fp8e4m3 terms summing to ~x."""
    terms = []
    r = x.astype(np.float32)
    for _ in range(levels):
        h = r.astype(f8)
        terms.append(h)
        r = r - h.astype(np.float32)
    return terms


def _host_prep(positions, W1, b1, W2, b2):
    pos = np.asarray(positions, dtype=np.float32)
    W1 = np.asarray(W1, dtype=np.float32)
    b1 = np.asarray(b1, dtype=np.float32)
    W2 = np.asarray(W2, dtype=np.float32)
    b2 = np.asarray(b2, dtype=np.float32)

    W1p, W1d = W1[:C], W1[C:]
    w2 = W2[:, 0]
    pos_idx = np.where(w2 >= 0)[0]
    neg_idx = np.where(w2 < 0)[0]
    np2, nn2 = len(pos_idx), len(neg_idx)
    A2 = np2 + nn2

    # permuted + |W2|-scaled channel coefficient matrices
    Wu2 = np.zeros((C, A2), np.float32)
    Wd2 = np.zeros((C, A2), np.float32)
    b1v = np.zeros((A2,), np.float32)
    for k, a in enumerate(list(pos_idx) + list(neg_idx)):
        g = abs(w2[a])
        Wu2[:, k] = g * (W1p[:, a] - W1d[:, a])
        Wd2[:, k] = g * W1d[:, a]
        b1v[k] = g * b1[a]

    u = pos @ Wu2 + b1v          # [B,T,N,A2]
    v = pos @ Wd2                # [B,T,N,A2]

    uhi = u.astype(bf16)
    ulo = (u - uhi.astype(np.float32)).astype(bf16)
    vhi = v.astype(bf16)
    vlo = (v - vhi.astype(np.float32)).astype(bf16)

    # channel-blocked column order within each t: first all positive
    # channels i-major (N*np2 cols), then all negative (N*nn2).
    NA = N * A2
    col_i = np.empty(NA, np.int64)
    col_ch = np.empty(NA, np.int64)
    c = 0
    for blk_base, blk_n in ((0, np2), (np2, nn2)):
        for i in range(N):
            for a in range(blk_n):
                col_i[c] = i
                col_ch[c] = blk_base + a
                c += 1

    # lhsT for the H matmul: rows [1; 1; v_hi(A2); v_lo(A2)] (per core)
    vT = np.empty((B, 2 * A2 + 2, T * N), dtype=bf16)
    vT[:, 0:2] = np.asarray(1.0, dtype=bf16)
    vT[:, 2:A2 + 2] = vhi.transpose(0, 3, 1, 2).reshape(B, A2, T * N)
    vT[:, A2 + 2:] = vlo.transpose(0, 3, 1, 2).reshape(B, A2, T * N)

    # per-t rhs rows for u (blocked column order): [T, 2, NA] bf16
    uflat = np.empty((B, T, 2, NA), dtype=bf16)
    uflat[:, :, 0] = uhi[:, :, col_i, col_ch]
    uflat[:, :, 1] = ulo[:, :, col_i, col_ch]

    # block-identity delta pattern, stacked twice (hi+lo rows) [2*A2, NA]
    delta1 = np.zeros((A2, NA), dtype=bf16)
    one_b = np.asarray(1.0, dtype=bf16)
    for c in range(NA):
        delta1[col_ch[c], c] = one_b
    delta = np.concatenate([delta1, delta1], axis=0)

    # dist^2 matmul operands, fp16 hi/lo split (K=10, cross terms kept)
    f16 = np.float16
    pos64 = pos.astype(np.float64)
    n2 = (pos64 ** 2).sum(-1)        # [B,T,N] (float64)
    px = pos64[..., 0].reshape(B, T * N)
    py = pos64[..., 1].reshape(B, T * N)
    n2f = n2.reshape(B, T * N)

    def hilo(x):
        hi = x.astype(f16)
        lo = (x - hi.astype(np.float64)).astype(f16)
        return hi, lo

    pxh, pxl = hilo(px)
    pyh, pyl = hilo(py)
    n2h, n2l = hilo(n2f)
    m2pxh, m2pxl = hilo(-2 * px)
    m2pyh, m2pyl = hilo(-2 * py)
    ones = np.ones_like(pxh)
    lhsTd = np.stack([pxh, pxh, pxl, pyh, pyh, pyl, ones, ones, n2h, n2l],
                     axis=1).astype(f16)                     # [B,10,T*N]
    rhsd = np.stack([m2pxh, m2pxl, m2pxh, m2pyh, m2pyl, m2pyh, n2h, n2l,
                     ones, ones], axis=1).astype(f16)

    # final-matmul rhs, fp16 hi/lo: per t 6 cols (pxh,pyh,1, pxl,pyl,0)
    pos3 = np.empty((B, N, T * 6), f16)
    p6 = pos3.reshape(B, N, T, 6)
    p6[..., 0] = pxh.reshape(B, T, N).transpose(0, 2, 1)
    p6[..., 1] = pyh.reshape(B, T, N).transpose(0, 2, 1)
    p6[..., 2] = 1.0
    p6[..., 3] = pxl.reshape(B, T, N).transpose(0, 2, 1)
    p6[..., 4] = pyl.reshape(B, T, N).transpose(0, 2, 1)
    p6[..., 5] = 0.0

    offd1 = (1.0 - np.eye(N)).astype(f16)
    offd = np.concatenate([offd1, offd1], axis=1)   # [N, 2N]

    return dict(vT=vT, uflat=uflat, delta=delta, lhsTd=lhsTd,
                rhsd=rhsd, pos3=pos3, offd=offd, A2=A2, np2=np2, nn2=nn2,
                b2=float(b2[0]))


def _build_program(A2, np2, nn2, b2val):
    import concourse.bacc as bacc
    import concourse.mybir as mybir
    import concourse.tile as tile

    f32 = mybir.dt.float32
    f16 = mybir.dt.float16
    bfl = mybir.dt.bfloat16
    K2 = 2 * A2 + 2
    Alu = mybir.AluOpType
    Act = mybir.ActivationFunctionType
    X = mybir.AxisListType.X

    NA = N * A2
    HALF = 64 * A2          # columns per PSUM half-tile

    nc = bacc.Bacc()

    vT_p = nc.declare_dram_parameter("vT", [K2, T * N], bfl, isOutput=False)
    uflat_p = nc.declare_dram_parameter("uflat", [T, 2, NA], bfl, isOutput=False)
    delta_p = nc.declare_dram_parameter("delta", [2 * A2, NA], bfl, isOutput=False)
    lhsTd_p = nc.declare_dram_parameter("lhsTd", [10, T * N], f16, isOutput=False)
    rhsd_p = nc.declare_dram_parameter("rhsd", [10, T * N], f16, isOutput=False)
    pos3_p = nc.declare_dram_parameter("pos3", [N, T * 6], f16, isOutput=False)
    offd_p = nc.declare_dram_parameter("offd", [N, 2 * N], f16, isOutput=False)
    out_p = nc.declare_dram_parameter("out", [T, N, C], f32, isOutput=True)

    with tile.TileContext(nc) as tc:
        with (
            tc.tile_pool(name="pers", bufs=1) as pers,
            tc.tile_pool(name="hpsum", bufs=3, space="PSUM") as hpsum,
            tc.tile_pool(name="dpsum", bufs=1, space="PSUM") as dpsum,
            tc.tile_pool(name="fpsum", bufs=1, space="PSUM") as fpsum,
            tc.tile_pool(name="work", bufs=4) as work,
            tc.tile_pool(name="wsmall", bufs=4) as wsmall,
        ):
            vT_s = pers.tile([K2, T * N], bfl, tag="vT")
            lhsTd_s = pers.tile([10, T * N], f16, tag="lhsTd")
            rhsd_s = pers.tile([10, T * N], f16, tag="rhsd")
            pos3_s = pers.tile([N, T * 6], f16, tag="pos3")
            offd_s = pers.tile([N, 2 * N], f16, tag="offd")
            rhsH = [pers.tile([K2, NA], bfl, tag=f"rhsH{i}", name=f"rhsH{i}")
                    for i in range(4)]

            nc.gpsimd.dma_start(vT_s[:], vT_p[:])
            nc.gpsimd.dma_start(lhsTd_s[:], lhsTd_p[:])
            nc.gpsimd.dma_start(rhsd_s[:], rhsd_p[:])
            nc.gpsimd.dma_start(pos3_s[:], pos3_p[:])
            nc.gpsimd.dma_start(offd_s[:], offd_p[:])
            for i in range(4):
                nc.gpsimd.dma_start(rhsH[i][2:2 * A2 + 2, :], delta_p[:])

            # persistent PSUM bank for the final row-sum matmuls: 8 cols/t
            # cols per t: [w@pxh, w@pyh, sum_w, w@pxl, w@pyl, 0, cnt, pad]
            pf = fpsum.tile([N, 8 * T], f32, tag="F")

            chunks = []
            off = 0
            while off < HALF:
                cn = min(512, HALF - off)
                chunks.append((off, cn))
                off += cn

            # per-4t-group state, emitted with deferral so no engine queue
            # head ever waits on a cross-engine producer:
            #   t = 4g..4g+3: H/relu/reduces/dist/mask/sub as data arrives
            #   t = 4g+4 (start): sigmoid(g) then w-muls(g) (inputs long done)
            #   t = 4g+5 (end):   final matmuls(g)
            gstate = {}

            def emit_sigmoid_w(g):
                st = gstate[g]
                nc.scalar.activation(st["att4"][:], st["pre4"][:], Act.Sigmoid,
                                     bias=b2val, scale=1.0)
                nc.gpsimd.tensor_mul(st["w4"][:, 0:2 * N],
                                     st["att4"][:, 0:2 * N], st["mask"][0][:])
                nc.gpsimd.tensor_mul(st["w4"][:, 2 * N:4 * N],
                                     st["att4"][:, 2 * N:4 * N], st["mask"][1][:])

            def emit_finals_one(g, dt_):
                st = gstate[g]
                tt = 4 * g + dt_
                s = dt_ * N
                nc.tensor.matmul(pf[:, 8 * tt:8 * tt + 6],
                                 st["w4"][:, s:s + N],
                                 pos3_s[:, 6 * tt:6 * tt + 6],
                                 start=True, stop=True)
                nc.tensor.matmul(
                    pf[:, 8 * tt + 6:8 * tt + 7],
                    st["mask"][dt_ // 2][:, (dt_ % 2) * N:(dt_ % 2 + 1) * N],
                    pos3_s[:, 6 * tt + 2:6 * tt + 3],
                    start=True, stop=True)

            for t in range(T):
                g2 = t % 2
                g4 = t % 4
                g = t // 4
                q = g4 // 2          # which 2t-pair inside the 4t group

                if g4 == 0:
                    if g - 1 in gstate:
                        emit_sigmoid_w(g - 1)
                    gstate[g] = {
                        "pre4": wsmall.tile([N, 4 * N], f16, tag="pre", name="pre4"),
                        "att4": wsmall.tile([N, 4 * N], f16, tag="att", name="att4"),
                        "w4": wsmall.tile([N, 4 * N], f16, tag="w", name="w4"),
                        "mask": [None, None],
                    }
                st = gstate[g]
                if g2 == 0:
                    attP2 = wsmall.tile([N, 2 * N], f32, tag="attP")
                    attM2 = wsmall.tile([N, 2 * N], f32, tag="attM")
                    st["mask"][q] = wsmall.tile([N, 2 * N], f16, tag="mask",
                                                name="mask2")
                    pd2 = dpsum.tile([N, 2 * N], f32, tag="D", name="pd2")

                rh = rhsH[t % 4]
                nc.sync.dma_start(rh[0:2, :], uflat_p[t])

                Rt = work.tile([N, NA], f16, tag="R")
                for h in range(2):
                    ph = hpsum.tile([N, HALF], f32, tag="H")
                    for (off, cn) in chunks:
                        nc.tensor.matmul(
                            ph[:, off:off + cn],
                            vT_s[:, t * N:(t + 1) * N],
                            rh[:, h * HALF + off:h * HALF + off + cn],
                            start=True, stop=True,
                        )
                    nc.scalar.activation(Rt[:, h * HALF:(h + 1) * HALF],
                                         ph[:], Act.Relu)

                # channel-blocked layout -> both reductions read contiguous
                # memory
                attP = attP2[:, g2 * N:(g2 + 1) * N]
                attM = attM2[:, g2 * N:(g2 + 1) * N]
                if np2 and nn2:
                    RP = Rt[:, 0:N * np2].rearrange("p (i a) -> p i a", a=np2)
                    RM = Rt[:, N * np2:NA].rearrange("p (i a) -> p i a", a=nn2)
                    nc.vector.tensor_reduce(attP, RP, axis=X, op=Alu.add)
                    nc.vector.tensor_reduce(attM, RM, axis=X, op=Alu.add)
                elif np2:
                    RP = Rt[:, 0:N * np2].rearrange("p (i a) -> p i a", a=np2)
                    nc.vector.tensor_reduce(attP, RP, axis=X, op=Alu.add)
                    nc.vector.tensor_scalar_mul(attM, attP, 0.0)
                else:
                    RM = Rt[:, N * np2:NA].rearrange("p (i a) -> p i a", a=nn2)
                    nc.vector.tensor_reduce(attM, RM, axis=X, op=Alu.add)
                    nc.vector.tensor_scalar_mul(attP, attM, 0.0)

                nc.tensor.matmul(pd2[:, g2 * N:(g2 + 1) * N],
                                 lhsTd_s[:, t * N:(t + 1) * N],
                                 rhsd_s[:, t * N:(t + 1) * N],
                                 start=True, stop=True)

                if g2 == 1:
                    # mask = (d^2 < R2) * offd; diagonal excluded by offd.
                    nc.vector.scalar_tensor_tensor(st["mask"][q][:], pd2[:],
                                                   R2, offd_s[:],
                                                   op0=Alu.is_lt, op1=Alu.mult)
                    nc.gpsimd.tensor_sub(st["pre4"][:, (g4 - 1) * N:(g4 + 1) * N],
                                         attP2[:], attM2[:])

                if g - 1 in gstate and g4 >= 0:
                    emit_finals_one(g - 1, g4)
                    if g4 == 3:
                        gstate.pop(g - 1)

            emit_sigmoid_w(T // 4 - 1)
            for dt_ in range(4):
                emit_finals_one(T // 4 - 1, dt_)
            gstate.pop(T // 4 - 1)

            # ---- tail: one pass over all T timesteps ----
            pf3 = pf[:].rearrange("p (t c) -> p t c", c=8)
            p3 = pos3_s[:].rearrange("p (t c) -> p t c", c=6)
            cntT = work.tile([N, T], f32, tag="cntT")
            rcpT = work.tile([N, T], f32, tag="rcpT")
            swT = work.tile([N, 2 * T], f32, tag="swT")
            outst = work.tile([N, 2 * T], f32, tag="outst")
            nc.vector.tensor_scalar_max(cntT[:], pf3[:, :, 6], 1e-6)
            nc.vector.reciprocal(rcpT[:], cntT[:])
            s3 = swT[:].rearrange("p (c t) -> p c t", c=2)
            o3 = outst[:].rearrange("p (t c) -> p t c", c=2)
            for c in range(2):
                nc.vector.tensor_add(s3[:, c], p3[:, :, c], p3[:, :, c + 3])
                nc.vector.tensor_mul(s3[:, c], pf3[:, :, 2], s3[:, c])
                nc.vector.tensor_sub(o3[:, :, c], pf3[:, :, c], s3[:, c])
                nc.vector.tensor_add(o3[:, :, c], o3[:, :, c], pf3[:, :, c + 3])
                nc.vector.tensor_mul(o3[:, :, c], o3[:, :, c], rcpT[:])
            nc.sync.dma_start(out_p[:].rearrange("t n c -> n t c"), outst[:])

    nc.compile()
    return nc


def kernel(positions, W1, b1, W2, b2, _trace=False, _trace_kwargs=None):
    from concourse.bass_utils import run_bass_kernel_spmd

    prep = _host_prep(positions, W1, b1, W2, b2)
    A2, np2, nn2, b2v = prep["A2"], prep["np2"], prep["nn2"], prep["b2"]

    key = (A2, np2, nn2, b2v)
    if key not in _CACHE:
        _CACHE[key] = _build_program(A2, np2, nn2, b2v)
    nc = _CACHE[key]

    in_maps = []
    for b in range(B):
        in_maps.append({
            "vT": np.ascontiguousarray(prep["vT"][b]),
            "uflat": np.ascontiguousarray(prep["uflat"][b]),
            "delta": prep["delta"],
            "lhsTd": np.ascontiguousarray(prep["lhsTd"][b]),
            "rhsd": np.ascontiguousarray(prep["rhsd"][b]),
            "pos3": np.ascontiguousarray(prep["pos3"][b]),
            "offd": prep["offd"],
        })

    kw = {}
    if _trace:
        kw["trace"] = True
        if _trace_kwargs:
            kw.update(_trace_kwargs)
    res = run_bass_kernel_spmd(nc, in_maps, list(range(B)), **kw)
    out = np.stack([r["out"] for r in res.results], axis=0).astype(np.float32)
    if _trace:
        return out, res
    return out
